# revision 16
# baseline (speedup 1.0000x reference)
"""CBOW (one-hot embedding lookup + mean + output matmul + softmax) on 8
Trainium2 NeuronCores, vocab-sharded end to end.

Full problem: batch [1024, 10, 32000] f32 one-hot, emb [32000, 128] f32,
w_out [128, 32000] f32 -> softmax(mean_c(batch @ emb) @ w_out) [1024, 32000].

Sharding: core i owns vocab columns [i*4000, (i+1)*4000). It receives
  batch_s [1024, 10, 4000] f32  (full batch, its vocab slice)
  emb_s   [4096, 128]      f32  (its emb rows, zero-padded 4000->4096)
  w_out_s [128, 4000]      f32  (its output-projection columns)
and produces out_s [1024, 4000] bf16 (its softmax columns; host concatenates
along vocab and upcasts to f32).

Batch rows run in 8 blocks of 128. Stage 1 streams each block's one-hot
slice as f32 via HWDGE on the sync engine (whose queue never waits on a
collective, so the stream issue front cannot stall), casts to bf16
on-chip (DVE + scalar engine for blocks 0-5; DVE-only for blocks 6-7 so
the scalar engine is free for the epilogue), then per 128-wide v-tile
sums the 10 context planes on the PE as REGULAR bf16 matmuls (lhsT=oh_c,
rhs=identity accumulates oh_c.T in fp32 PSUM) giving sT[v, b];
sumT_bb[d, b] += emb_tile.T @ sT over the core's 32 v-tiles.

Cross-core reduction: TWO waves (blocks 0-5, 6-7), 4 collectives total,
all triggered from gpsimd which does nothing else (a trigger blocks its
issuing engine for the collective's duration). Softmax uses a
recompute-with-bias scheme that needs no stored activations: stage 2a
runs the logits matmul once, computing only the row-sums of exp(x/C)
(scalar-engine activation accum_out; the exp values land in a discarded
scratch tile); after the wave's tiny denominator AllReduce, stage 2b
re-runs the matmul and emits exp(x/C - ln S) directly as bf16 via the
activation's per-partition bias - no reciprocal, no rescale multiplies,
no [128 x 4000] exp buffers alive across the pipeline. Wave 0's entire
stage 2 hides under blocks 6-7's streaming; only wave 1's chain
(avg-AllReduce, 16 matmuls + exps, den-AllReduce, 16 matmuls + biased
exps) runs after the stream.
"""

from contextlib import ExitStack

import numpy as np

import concourse.bass as bass
import concourse.tile as tile
from concourse import bacc, masks, mybir
from concourse._compat import with_exitstack

F32 = mybir.dt.float32
BF16 = mybir.dt.bfloat16
AX = mybir.AxisListType
AF = mybir.ActivationFunctionType

B_FULL, C, V, D = 1024, 10, 32000, 128
N_CORES = 8
VS = V // N_CORES          # 4000 vocab columns per core
VS_PAD = 4096              # emb rows padded to a multiple of 128
N_TILES = VS_PAD // 128    # 32 v-tiles (last is 32 valid rows)
BB = 128                   # batch rows per block
N_BB = B_FULL // BB        # 8 blocks
VC = 1024                  # one-hot v-chunk (chunks: 1024,1024,1024,928)
NC2 = 512                  # stage-2 logits chunk
C_DVE = 5                  # context planes cast on DVE (rest on scalar)

WAVES = [(0, 6), (6, 8)]


@with_exitstack
def _cbow_kernel(ctx: ExitStack, tc, out, batch, emb, w_out):
    nc = tc.nc
    Bs, Cs, Vs = batch.shape
    assert Bs == B_FULL and Cs == C and Vs == VS
    rg = [list(range(N_CORES))]
    n_vc = (Vs + VC - 1) // VC
    n_nc = (Vs + NC2 - 1) // NC2

    const_pool = ctx.enter_context(tc.tile_pool(name="const", bufs=1))
    ident = const_pool.tile([128, 128], BF16)
    masks.make_identity(nc, ident[:])

    eb_pool = ctx.enter_context(tc.tile_pool(name="eb", bufs=1))
    eb = eb_pool.tile([128, N_TILES, 128], F32)
    nc.sync.dma_start(eb[:], emb.rearrange("(n p) d -> p n d", p=128))
    wo_pool = ctx.enter_context(tc.tile_pool(name="wo", bufs=1))
    wo = wo_pool.tile([128, VS], F32)
    nc.sync.dma_start(wo[:], w_out)

    oh32_pool = ctx.enter_context(tc.tile_pool(name="oh32", bufs=2))
    ohb_pool = ctx.enter_context(tc.tile_pool(name="ohb", bufs=3))
    sT_pool = ctx.enter_context(tc.tile_pool(name="sT", bufs=4))
    sTps_pool = ctx.enter_context(tc.tile_pool(name="sTps", bufs=3, space="PSUM"))
    acc_pool = ctx.enter_context(tc.tile_pool(name="acc", bufs=2, space="PSUM"))
    avgsb_pool = ctx.enter_context(tc.tile_pool(name="avgsb", bufs=1))
    avgg_pool = ctx.enter_context(tc.tile_pool(name="avgg", bufs=1))
    osb_pool = ctx.enter_context(tc.tile_pool(name="osb", bufs=2))
    scr_pool = ctx.enter_context(tc.tile_pool(name="scr", bufs=2))
    lgps_pool = ctx.enter_context(tc.tile_pool(name="lgps", bufs=3, space="PSUM"))
    stat_pool = ctx.enter_context(tc.tile_pool(name="stat", bufs=2))
    dram = ctx.enter_context(tc.tile_pool(name="dram", bufs=2, space="DRAM"))

    wstate = {}
    for wi, (w0, w1) in enumerate(WAVES):
        cols = (w1 - w0) * BB
        wstate[wi] = {
            "w0": w0,
            "w1": w1,
            "avg_sb": avgsb_pool.tile(
                [128, cols], F32, tag=f"avgsb{wi}", name=f"avgsb{wi}"
            ),
            "den_sb": stat_pool.tile(
                [128, w1 - w0], F32, tag=f"densb{wi}", name=f"densb{wi}"
            ),
        }

    bb2wave = {}
    for wi, (w0, w1) in enumerate(WAVES):
        for bb in range(w0, w1):
            bb2wave[bb] = wi

    def stage1_chunk(bb, j, avgT_ps):
        b0 = bb * BB
        v0 = j * VC
        vc = min(VC, Vs - v0)
        oh32 = oh32_pool.tile([128, Cs, VC], F32, tag="oh32")
        nc.sync.dma_start(
            oh32[:, :, :vc], batch[b0 : b0 + BB, :, v0 : v0 + vc]
        )
        ohb = ohb_pool.tile([128, Cs, VC], BF16, tag="ohb")
        if bb >= WAVES[-1][0]:
            # scalar engine is busy with wave-0's epilogue here
            nc.vector.tensor_copy(ohb[:, :, :vc], oh32[:, :, :vc])
        else:
            nc.vector.tensor_copy(ohb[:, :C_DVE, :vc], oh32[:, :C_DVE, :vc])
            nc.scalar.copy(ohb[:, C_DVE:, :vc], oh32[:, C_DVE:, :vc])
        nt = (vc + 127) // 128
        for t in range(nt):
            toff = t * 128
            tw = min(128, vc - toff)
            g = j * (VC // 128) + t
            sT_ps = sTps_pool.tile([128, BB], F32, tag="sTps")
            for c in range(Cs):
                nc.tensor.matmul(
                    sT_ps[:tw],
                    lhsT=ohb[:, c, toff : toff + tw],
                    rhs=ident[:],
                    start=(c == 0),
                    stop=(c == Cs - 1),
                )
            sT = sT_pool.tile([128, BB], F32, tag="sT")
            nc.vector.tensor_copy(sT[:tw], sT_ps[:tw])
            nc.tensor.matmul(
                avgT_ps[:],
                lhsT=eb[:tw, g, :],
                rhs=sT[:tw],
                start=(g == 0),
                stop=(g == N_TILES - 1),
            )

    def avg_ar(wi):
        """Bounce the wave's context-sums to DRAM, AllReduce, read back."""
        s = wstate[wi]
        cols = (s["w1"] - s["w0"]) * BB
        cc_in = dram.tile([128, cols], F32, tag=f"cc_in{wi}", bufs=1)
        cc_out = dram.tile(
            [128, cols], F32, tag=f"cc_out{wi}", addr_space="Shared", bufs=1
        )
        nc.sync.dma_start(cc_in[:], s["avg_sb"][:])
        nc.gpsimd.collective_compute(
            "AllReduce",
            mybir.AluOpType.add,
            replica_groups=rg,
            ins=[cc_in.opt()],
            outs=[cc_out.opt()],
        )
        avg_g = avgg_pool.tile([128, cols], F32, tag=f"avgg{wi}")
        nc.gpsimd.dma_start(avg_g[:], cc_out[:])
        s["avg_g"] = avg_g

    def den_ar(wi):
        s = wstate[wi]
        nb = s["w1"] - s["w0"]
        cc_in = dram.tile([128, nb], F32, tag=f"cc2_in{wi}", bufs=1)
        cc_out = dram.tile(
            [128, nb], F32, tag=f"cc2_out{wi}", addr_space="Shared", bufs=1
        )
        nc.sync.dma_start(cc_in[:], s["den_sb"][:])
        nc.gpsimd.collective_compute(
            "AllReduce",
            mybir.AluOpType.add,
            replica_groups=rg,
            ins=[cc_in.opt()],
            outs=[cc_out.opt()],
        )
        den_g = stat_pool.tile([128, nb], F32, tag=f"deng{wi}")
        nc.gpsimd.dma_start(den_g[:], cc_out[:])
        s["den_g"] = den_g

    def stage2a(bb):
        """Logits pass 1: only the exp row-sums survive (scratch discard)."""
        wi = bb2wave[bb]
        s = wstate[wi]
        slot = bb - s["w0"]
        avg_g = s["avg_g"]
        sums = stat_pool.tile([128, n_nc], F32, tag="sums")
        for k in range(n_nc):
            n0 = k * NC2
            nw = min(NC2, Vs - n0)
            lg_ps = lgps_pool.tile([128, NC2], F32, tag="lgps")
            nc.tensor.matmul(
                lg_ps[:, :nw],
                lhsT=avg_g[:, slot * BB : (slot + 1) * BB],
                rhs=wo[:, n0 : n0 + nw],
                start=True,
                stop=True,
            )
            scr = scr_pool.tile([128, NC2], BF16, tag="scr")
            nc.scalar.activation(
                scr[:, :nw],
                lg_ps[:, :nw],
                AF.Exp,
                scale=1.0 / Cs,
                accum_out=sums[:, k : k + 1],
            )
        scr2 = stat_pool.tile([128, n_nc], F32, tag="scr2")
        nc.scalar.activation(
            scr2[:, :n_nc],
            sums[:, :n_nc],
            AF.Copy,
            accum_out=s["den_sb"][:, slot : slot + 1],
        )

    def stage2b(wi):
        """Logits pass 2: out = exp(x/C - ln S), written straight to bf16."""
        s = wstate[wi]
        nb = s["w1"] - s["w0"]
        lnS = stat_pool.tile([128, nb], F32, tag=f"lnS{wi}")
        nc.scalar.activation(lnS[:], s["den_g"][:], AF.Ln)
        nlnS = stat_pool.tile([128, nb], F32, tag=f"nlnS{wi}")
        nc.vector.tensor_scalar_mul(nlnS[:], lnS[:], -1.0)
        for bb in range(s["w0"], s["w1"]):
            slot = bb - s["w0"]
            osb = osb_pool.tile([128, VS], BF16, tag="osb")
            for k in range(n_nc):
                n0 = k * NC2
                nw = min(NC2, Vs - n0)
                lg_ps = lgps_pool.tile([128, NC2], F32, tag="lgps")
                nc.tensor.matmul(
                    lg_ps[:, :nw],
                    lhsT=s["avg_g"][:, slot * BB : (slot + 1) * BB],
                    rhs=wo[:, n0 : n0 + nw],
                    start=True,
                    stop=True,
                )
                nc.scalar.activation(
                    osb[:, n0 : n0 + nw],
                    lg_ps[:, :nw],
                    AF.Exp,
                    scale=1.0 / Cs,
                    bias=nlnS[:, slot : slot + 1],
                )
            b0 = bb * BB
            nc.scalar.dma_start(out[b0 : b0 + BB, :], osb[:])

    # wave 0's reduction fires as soon as block 5's sums are parked; its
    # stage 2 spreads across blocks 6-7's streaming windows.
    events = {
        (6, 2): [lambda: avg_ar(0)],
        (6, 3): [lambda: stage2a(0)],
        (7, 0): [lambda: stage2a(1), lambda: stage2a(2)],
        (7, 1): [lambda: stage2a(3)],
        (7, 2): [lambda: stage2a(4)],
        (7, 3): [lambda: stage2a(5)],
    }

    for bb in range(N_BB):
        avgT_ps = acc_pool.tile([128, BB], F32, tag="acc")
        for j in range(n_vc):
            stage1_chunk(bb, j, avgT_ps)
            for fn in events.get((bb, j), []):
                fn()
        wi = bb2wave[bb]
        s = wstate[wi]
        slot = bb - s["w0"]
        nc.vector.tensor_copy(
            s["avg_sb"][:, slot * BB : (slot + 1) * BB], avgT_ps[:]
        )

    den_ar(0)
    stage2b(0)
    avg_ar(1)
    stage2a(6)
    stage2a(7)
    den_ar(1)
    stage2b(1)


def build(num_devices=N_CORES):
    nc = bacc.Bacc(
        "TRN2",
        target_bir_lowering=False,
        debug=False,
        num_devices=num_devices,
        num_swdge_queues=4,
    )
    batch = nc.dram_tensor(
        "batch", [B_FULL, C, VS], F32, kind="ExternalInput"
    ).ap()
    emb = nc.dram_tensor("emb", [VS_PAD, D], F32, kind="ExternalInput").ap()
    w_out = nc.dram_tensor("w_out", [D, VS], F32, kind="ExternalInput").ap()
    out = nc.dram_tensor("out", [B_FULL, VS], BF16, kind="ExternalOutput").ap()
    with tile.TileContext(nc) as tc:
        _cbow_kernel(tc, out, batch, emb, w_out)
    nc.compile()
    return nc


_NC = None


def _build_cached():
    global _NC
    if _NC is None:
        _NC = build()
    return _NC


def _run(batch, emb, w_out, trace=False, **kwargs):
    from concourse.bass_utils import run_bass_kernel_spmd

    nc = _build_cached()
    batch = np.ascontiguousarray(np.asarray(batch, dtype=np.float32))
    emb = np.asarray(emb, dtype=np.float32)
    w_out = np.asarray(w_out, dtype=np.float32)
    in_maps = []
    for i in range(N_CORES):
        v0 = i * VS
        emb_pad = np.zeros((VS_PAD, D), dtype=np.float32)
        emb_pad[:VS] = emb[v0 : v0 + VS]
        in_maps.append(
            {
                "batch": np.ascontiguousarray(batch[:, :, v0 : v0 + VS]),
                "emb": emb_pad,
                "w_out": np.ascontiguousarray(w_out[:, v0 : v0 + VS]),
            }
        )
    res = run_bass_kernel_spmd(
        nc, in_maps, core_ids=list(range(N_CORES)), trace=trace, **kwargs
    )
    out = np.concatenate(
        [r["out"].astype(np.float32) for r in res.results], axis=1
    )
    return out, res


def kernel(batch, emb, w_out):
    out, _ = _run(batch, emb, w_out, trace=False)
    return out


# revision 18
# speedup vs baseline: 1.0114x; 1.0114x over previous
"""CBOW (one-hot embedding lookup + mean + output matmul + softmax) on 8
Trainium2 NeuronCores, vocab-sharded end to end.

Full problem: batch [1024, 10, 32000] f32 one-hot, emb [32000, 128] f32,
w_out [128, 32000] f32 -> softmax(mean_c(batch @ emb) @ w_out) [1024, 32000].

Sharding: core i owns vocab columns [i*4000, (i+1)*4000). It receives
  batch_s [1024, 10, 4000] f32  (full batch, its vocab slice)
  emb_s   [4096, 128]      f32  (its emb rows, zero-padded 4000->4096)
  w_out_s [128, 4000]      f32  (its output-projection columns)
and produces out_s [1024, 4000] bf16 (its softmax columns; host concatenates
along vocab and upcasts to f32).

Batch rows run in 8 blocks of 128. Stage 1 streams each block's one-hot
slice as f32 via HWDGE on the sync engine (whose queue never waits on a
collective, so the stream issue front cannot stall), casts to bf16
on-chip (DVE + scalar engine for blocks 0-5; DVE-only for blocks 6-7 so
the scalar engine is free for the epilogue), then per 128-wide v-tile
sums the 10 context planes on the PE as REGULAR bf16 matmuls (lhsT=oh_c,
rhs=identity accumulates oh_c.T in fp32 PSUM) giving sT[v, b];
sumT_bb[d, b] += emb_tile.T @ sT over the core's 32 v-tiles.

Cross-core reduction: TWO waves (blocks 0-5, 6-7), 4 collectives total,
all triggered from gpsimd which does nothing else (a trigger blocks its
issuing engine for the collective's duration). Softmax uses a
recompute-with-bias scheme that needs no stored activations: stage 2a
runs the logits matmul once, computing only the row-sums of exp(x/C)
(scalar-engine activation accum_out; the exp values land in a discarded
scratch tile); after the wave's tiny denominator AllReduce, stage 2b
re-runs the matmul and emits exp(x/C - ln S) directly as bf16 via the
activation's per-partition bias - no reciprocal, no rescale multiplies,
no [128 x 4000] exp buffers alive across the pipeline. Wave 0's entire
stage 2 hides under blocks 6-7's streaming; only wave 1's chain
(avg-AllReduce, 16 matmuls + exps, den-AllReduce, 16 matmuls + biased
exps) runs after the stream.
"""

from contextlib import ExitStack

import numpy as np

import concourse.bass as bass
import concourse.tile as tile
from concourse import bacc, masks, mybir
from concourse._compat import with_exitstack

F32 = mybir.dt.float32
BF16 = mybir.dt.bfloat16
AX = mybir.AxisListType
AF = mybir.ActivationFunctionType

B_FULL, C, V, D = 1024, 10, 32000, 128
N_CORES = 8
VS = V // N_CORES          # 4000 vocab columns per core
VS_PAD = 4096              # emb rows padded to a multiple of 128
N_TILES = VS_PAD // 128    # 32 v-tiles (last is 32 valid rows)
BB = 128                   # batch rows per block
N_BB = B_FULL // BB        # 8 blocks
VC = 1024                  # one-hot v-chunk (chunks: 1024,1024,1024,928)
NC2 = 512                  # stage-2 logits chunk
C_DVE = 5                  # context planes cast on DVE (rest on scalar)

WAVES = [(0, 5), (5, 7), (7, 8)]


@with_exitstack
def _cbow_kernel(ctx: ExitStack, tc, out, batch, emb, w_out):
    nc = tc.nc
    Bs, Cs, Vs = batch.shape
    assert Bs == B_FULL and Cs == C and Vs == VS
    rg = [list(range(N_CORES))]
    n_vc = (Vs + VC - 1) // VC
    n_nc = (Vs + NC2 - 1) // NC2

    const_pool = ctx.enter_context(tc.tile_pool(name="const", bufs=1))
    ident = const_pool.tile([128, 128], BF16)
    masks.make_identity(nc, ident[:])

    eb_pool = ctx.enter_context(tc.tile_pool(name="eb", bufs=1))
    eb = eb_pool.tile([128, N_TILES, 128], F32)
    nc.sync.dma_start(eb[:], emb.rearrange("(n p) d -> p n d", p=128))
    wo_pool = ctx.enter_context(tc.tile_pool(name="wo", bufs=1))
    wo = wo_pool.tile([128, VS], F32)

    oh32_pool = ctx.enter_context(tc.tile_pool(name="oh32", bufs=2))
    ohb_pool = ctx.enter_context(tc.tile_pool(name="ohb", bufs=3))
    sT_pool = ctx.enter_context(tc.tile_pool(name="sT", bufs=4))
    sTps_pool = ctx.enter_context(tc.tile_pool(name="sTps", bufs=3, space="PSUM"))
    acc_pool = ctx.enter_context(tc.tile_pool(name="acc", bufs=2, space="PSUM"))
    avgsb_pool = ctx.enter_context(tc.tile_pool(name="avgsb", bufs=1))
    avgg_pool = ctx.enter_context(tc.tile_pool(name="avgg", bufs=1))
    osb_pool = ctx.enter_context(tc.tile_pool(name="osb", bufs=2))
    scr_pool = ctx.enter_context(tc.tile_pool(name="scr", bufs=2))
    lgps_pool = ctx.enter_context(tc.tile_pool(name="lgps", bufs=3, space="PSUM"))
    stat_pool = ctx.enter_context(tc.tile_pool(name="stat", bufs=2))
    dram = ctx.enter_context(tc.tile_pool(name="dram", bufs=2, space="DRAM"))

    wstate = {}
    for wi, (w0, w1) in enumerate(WAVES):
        cols = (w1 - w0) * BB
        wstate[wi] = {
            "w0": w0,
            "w1": w1,
            "avg_sb": avgsb_pool.tile(
                [128, cols], F32, tag=f"avgsb{wi}", name=f"avgsb{wi}"
            ),
            "den_sb": stat_pool.tile(
                [128, w1 - w0], F32, tag=f"densb{wi}", name=f"densb{wi}"
            ),
        }

    bb2wave = {}
    for wi, (w0, w1) in enumerate(WAVES):
        for bb in range(w0, w1):
            bb2wave[bb] = wi

    def stage1_chunk(bb, j, avgT_ps):
        b0 = bb * BB
        v0 = j * VC
        vc = min(VC, Vs - v0)
        oh32 = oh32_pool.tile([128, Cs, VC], F32, tag="oh32")
        nc.sync.dma_start(
            oh32[:, :, :vc], batch[b0 : b0 + BB, :, v0 : v0 + vc]
        )
        ohb = ohb_pool.tile([128, Cs, VC], BF16, tag="ohb")
        if bb >= WAVES[-1][0]:
            # scalar engine is busy with wave-0's epilogue here
            nc.vector.tensor_copy(ohb[:, :, :vc], oh32[:, :, :vc])
        else:
            nc.vector.tensor_copy(ohb[:, :C_DVE, :vc], oh32[:, :C_DVE, :vc])
            nc.scalar.copy(ohb[:, C_DVE:, :vc], oh32[:, C_DVE:, :vc])
        nt = (vc + 127) // 128
        for t in range(nt):
            toff = t * 128
            tw = min(128, vc - toff)
            g = j * (VC // 128) + t
            sT_ps = sTps_pool.tile([128, BB], F32, tag="sTps")
            for c in range(Cs):
                nc.tensor.matmul(
                    sT_ps[:tw],
                    lhsT=ohb[:, c, toff : toff + tw],
                    rhs=ident[:],
                    start=(c == 0),
                    stop=(c == Cs - 1),
                )
            sT = sT_pool.tile([128, BB], F32, tag="sT")
            nc.vector.tensor_copy(sT[:tw], sT_ps[:tw])
            nc.tensor.matmul(
                avgT_ps[:],
                lhsT=eb[:tw, g, :],
                rhs=sT[:tw],
                start=(g == 0),
                stop=(g == N_TILES - 1),
            )

    def avg_ar(wi):
        """Bounce the wave's context-sums to DRAM, AllReduce, read back."""
        s = wstate[wi]
        cols = (s["w1"] - s["w0"]) * BB
        cc_in = dram.tile([128, cols], F32, tag=f"cc_in{wi}", bufs=1)
        cc_out = dram.tile(
            [128, cols], F32, tag=f"cc_out{wi}", addr_space="Shared", bufs=1
        )
        nc.sync.dma_start(cc_in[:], s["avg_sb"][:])
        nc.gpsimd.collective_compute(
            "AllReduce",
            mybir.AluOpType.add,
            replica_groups=rg,
            ins=[cc_in.opt()],
            outs=[cc_out.opt()],
        )
        avg_g = avgg_pool.tile([128, cols], F32, tag=f"avgg{wi}")
        nc.gpsimd.dma_start(avg_g[:], cc_out[:])
        s["avg_g"] = avg_g

    def den_ar(wi):
        s = wstate[wi]
        nb = s["w1"] - s["w0"]
        cc_in = dram.tile([128, nb], F32, tag=f"cc2_in{wi}", bufs=1)
        cc_out = dram.tile(
            [128, nb], F32, tag=f"cc2_out{wi}", addr_space="Shared", bufs=1
        )
        nc.sync.dma_start(cc_in[:], s["den_sb"][:])
        nc.gpsimd.collective_compute(
            "AllReduce",
            mybir.AluOpType.add,
            replica_groups=rg,
            ins=[cc_in.opt()],
            outs=[cc_out.opt()],
        )
        den_g = stat_pool.tile([128, nb], F32, tag=f"deng{wi}")
        nc.gpsimd.dma_start(den_g[:], cc_out[:])
        s["den_g"] = den_g

    def stage2a(bb):
        """Logits pass 1: only the exp row-sums survive (scratch discard)."""
        wi = bb2wave[bb]
        s = wstate[wi]
        slot = bb - s["w0"]
        avg_g = s["avg_g"]
        sums = stat_pool.tile([128, n_nc], F32, tag="sums")
        for k in range(n_nc):
            n0 = k * NC2
            nw = min(NC2, Vs - n0)
            lg_ps = lgps_pool.tile([128, NC2], F32, tag="lgps")
            nc.tensor.matmul(
                lg_ps[:, :nw],
                lhsT=avg_g[:, slot * BB : (slot + 1) * BB],
                rhs=wo[:, n0 : n0 + nw],
                start=True,
                stop=True,
            )
            scr = scr_pool.tile([128, NC2], BF16, tag="scr")
            nc.scalar.activation(
                scr[:, :nw],
                lg_ps[:, :nw],
                AF.Exp,
                scale=1.0 / Cs,
                accum_out=sums[:, k : k + 1],
            )
        scr2 = stat_pool.tile([128, n_nc], F32, tag="scr2")
        nc.scalar.activation(
            scr2[:, :n_nc],
            sums[:, :n_nc],
            AF.Copy,
            accum_out=s["den_sb"][:, slot : slot + 1],
        )

    def stage2b(wi):
        """Logits pass 2: out = exp(x/C - ln S), written straight to bf16."""
        s = wstate[wi]
        nb = s["w1"] - s["w0"]
        lnS = stat_pool.tile([128, nb], F32, tag=f"lnS{wi}")
        nc.scalar.activation(lnS[:], s["den_g"][:], AF.Ln)
        nlnS = stat_pool.tile([128, nb], F32, tag=f"nlnS{wi}")
        nc.vector.tensor_scalar_mul(nlnS[:], lnS[:], -1.0)
        for bb in range(s["w0"], s["w1"]):
            slot = bb - s["w0"]
            osb = osb_pool.tile([128, VS], BF16, tag="osb")
            for k in range(n_nc):
                n0 = k * NC2
                nw = min(NC2, Vs - n0)
                lg_ps = lgps_pool.tile([128, NC2], F32, tag="lgps")
                nc.tensor.matmul(
                    lg_ps[:, :nw],
                    lhsT=s["avg_g"][:, slot * BB : (slot + 1) * BB],
                    rhs=wo[:, n0 : n0 + nw],
                    start=True,
                    stop=True,
                )
                nc.scalar.activation(
                    osb[:, n0 : n0 + nw],
                    lg_ps[:, :nw],
                    AF.Exp,
                    scale=1.0 / Cs,
                    bias=nlnS[:, slot : slot + 1],
                )
            b0 = bb * BB
            nc.scalar.dma_start(out[b0 : b0 + BB, :], osb[:])

    # weight loads hide inside the first chunks; each wave's reduction
    # fires as soon as its last block's sums are parked (at ~16 us/chunk
    # arrival pace), and its stage 2 spreads over later blocks' windows.
    events = {
        (0, 1): [lambda: nc.sync.dma_start(wo[:], w_out)],
        (5, 1): [lambda: avg_ar(0)],
        (5, 2): [lambda: stage2a(0)],
        (5, 3): [lambda: stage2a(1)],
        (6, 0): [lambda: stage2a(2)],
        (6, 1): [lambda: stage2a(3)],
        (6, 2): [lambda: stage2a(4)],
        (6, 3): [lambda: den_ar(0)],
        (7, 0): [lambda: stage2b(0)],
        (7, 1): [lambda: avg_ar(1), lambda: stage2a(5)],
        (7, 2): [lambda: stage2a(6)],
        (7, 3): [lambda: den_ar(1)],
    }

    for bb in range(N_BB):
        avgT_ps = acc_pool.tile([128, BB], F32, tag="acc")
        for j in range(n_vc):
            stage1_chunk(bb, j, avgT_ps)
            for fn in events.get((bb, j), []):
                fn()
        wi = bb2wave[bb]
        s = wstate[wi]
        slot = bb - s["w0"]
        nc.vector.tensor_copy(
            s["avg_sb"][:, slot * BB : (slot + 1) * BB], avgT_ps[:]
        )

    stage2b(1)
    avg_ar(2)
    stage2a(7)
    den_ar(2)
    stage2b(2)


def build(num_devices=N_CORES):
    nc = bacc.Bacc(
        "TRN2",
        target_bir_lowering=False,
        debug=False,
        num_devices=num_devices,
        num_swdge_queues=4,
    )
    batch = nc.dram_tensor(
        "batch", [B_FULL, C, VS], F32, kind="ExternalInput"
    ).ap()
    emb = nc.dram_tensor("emb", [VS_PAD, D], F32, kind="ExternalInput").ap()
    w_out = nc.dram_tensor("w_out", [D, VS], F32, kind="ExternalInput").ap()
    out = nc.dram_tensor("out", [B_FULL, VS], BF16, kind="ExternalOutput").ap()
    with tile.TileContext(nc) as tc:
        _cbow_kernel(tc, out, batch, emb, w_out)
    nc.compile()
    return nc


_NC = None


def _build_cached():
    global _NC
    if _NC is None:
        _NC = build()
    return _NC


def _run(batch, emb, w_out, trace=False, **kwargs):
    from concourse.bass_utils import run_bass_kernel_spmd

    nc = _build_cached()
    batch = np.ascontiguousarray(np.asarray(batch, dtype=np.float32))
    emb = np.asarray(emb, dtype=np.float32)
    w_out = np.asarray(w_out, dtype=np.float32)
    in_maps = []
    for i in range(N_CORES):
        v0 = i * VS
        emb_pad = np.zeros((VS_PAD, D), dtype=np.float32)
        emb_pad[:VS] = emb[v0 : v0 + VS]
        in_maps.append(
            {
                "batch": np.ascontiguousarray(batch[:, :, v0 : v0 + VS]),
                "emb": emb_pad,
                "w_out": np.ascontiguousarray(w_out[:, v0 : v0 + VS]),
            }
        )
    res = run_bass_kernel_spmd(
        nc, in_maps, core_ids=list(range(N_CORES)), trace=trace, **kwargs
    )
    out = np.concatenate(
        [r["out"].astype(np.float32) for r in res.results], axis=1
    )
    return out, res


def kernel(batch, emb, w_out):
    out, _ = _run(batch, emb, w_out, trace=False)
    return out


# revision 19
# speedup vs baseline: 1.0845x; 1.0723x over previous
"""CBOW (one-hot embedding lookup + mean + output matmul + softmax) on 8
Trainium2 NeuronCores, vocab-sharded end to end.

Full problem: batch [1024, 10, 32000] f32 one-hot, emb [32000, 128] f32,
w_out [128, 32000] f32 -> softmax(mean_c(batch @ emb) @ w_out) [1024, 32000].

Sharding: core i owns vocab columns [i*4000, (i+1)*4000). It receives
  batch_s [1024, 10, 4000] f32  (full batch, its vocab slice)
  emb_s   [4096, 128]      f32  (its emb rows, zero-padded 4000->4096)
  w_out_s [128, 4000]      f32  (its output-projection columns)
and produces out_s [1024, 4000] bf16 (its softmax columns; host concatenates
along vocab and upcasts to f32).

Batch rows run in 8 blocks of 128. Stage 1 streams each block's one-hot
slice as f32 via HWDGE on the sync engine (whose queue never waits on a
collective, so the stream issue front cannot stall), casts to bf16
on-chip (DVE + scalar engine for blocks 0-5; DVE-only for blocks 6-7 so
the scalar engine is free for the epilogue), then per 128-wide v-tile
sums the 10 context planes on the PE as REGULAR bf16 matmuls (lhsT=oh_c,
rhs=identity accumulates oh_c.T in fp32 PSUM) giving sT[v, b];
sumT_bb[d, b] += emb_tile.T @ sT over the core's 32 v-tiles.

Cross-core reduction: TWO waves (blocks 0-5, 6-7), 4 collectives total,
all triggered from gpsimd which does nothing else (a trigger blocks its
issuing engine for the collective's duration). Softmax uses a
recompute-with-bias scheme that needs no stored activations: stage 2a
runs the logits matmul once, computing only the row-sums of exp(x/C)
(scalar-engine activation accum_out; the exp values land in a discarded
scratch tile); after the wave's tiny denominator AllReduce, stage 2b
re-runs the matmul and emits exp(x/C - ln S) directly as bf16 via the
activation's per-partition bias - no reciprocal, no rescale multiplies,
no [128 x 4000] exp buffers alive across the pipeline. Wave 0's entire
stage 2 hides under blocks 6-7's streaming; only wave 1's chain
(avg-AllReduce, 16 matmuls + exps, den-AllReduce, 16 matmuls + biased
exps) runs after the stream.
"""

from contextlib import ExitStack

import numpy as np

import concourse.bass as bass
import concourse.tile as tile
from concourse import bacc, masks, mybir
from concourse._compat import with_exitstack

F32 = mybir.dt.float32
BF16 = mybir.dt.bfloat16
AX = mybir.AxisListType
AF = mybir.ActivationFunctionType

B_FULL, C, V, D = 1024, 10, 32000, 128
N_CORES = 8
VS = V // N_CORES          # 4000 vocab columns per core
VS_PAD = 4096              # emb rows padded to a multiple of 128
N_TILES = VS_PAD // 128    # 32 v-tiles (last is 32 valid rows)
BB = 128                   # batch rows per block
N_BB = B_FULL // BB        # 8 blocks
VC = 1024                  # one-hot v-chunk (chunks: 1024,1024,1024,928)
NC2 = 512                  # stage-2 logits chunk
C_DVE = 5                  # context planes cast on DVE (rest on scalar)

DGROUPS = [(0, 4), (4, 6), (6, 8)]  # denominator all-reduce groups


@with_exitstack
def _cbow_kernel(ctx: ExitStack, tc, out, batch, emb, w_out):
    nc = tc.nc
    Bs, Cs, Vs = batch.shape
    assert Bs == B_FULL and Cs == C and Vs == VS
    rg = [list(range(N_CORES))]
    n_vc = (Vs + VC - 1) // VC
    n_nc = (Vs + NC2 - 1) // NC2

    const_pool = ctx.enter_context(tc.tile_pool(name="const", bufs=1))
    ident = const_pool.tile([128, 128], BF16)
    masks.make_identity(nc, ident[:])

    eb_pool = ctx.enter_context(tc.tile_pool(name="eb", bufs=1))
    eb = eb_pool.tile([128, N_TILES, 128], F32)
    nc.sync.dma_start(eb[:], emb.rearrange("(n p) d -> p n d", p=128))
    wo_pool = ctx.enter_context(tc.tile_pool(name="wo", bufs=1))
    wo = wo_pool.tile([128, VS], F32)

    oh32_pool = ctx.enter_context(tc.tile_pool(name="oh32", bufs=2))
    ohb_pool = ctx.enter_context(tc.tile_pool(name="ohb", bufs=3))
    sT_pool = ctx.enter_context(tc.tile_pool(name="sT", bufs=4))
    sTps_pool = ctx.enter_context(tc.tile_pool(name="sTps", bufs=3, space="PSUM"))
    acc_pool = ctx.enter_context(tc.tile_pool(name="acc", bufs=2, space="PSUM"))
    avgsb_pool = ctx.enter_context(tc.tile_pool(name="avgsb", bufs=2))
    avgg_pool = ctx.enter_context(tc.tile_pool(name="avgg", bufs=8))
    osb_pool = ctx.enter_context(tc.tile_pool(name="osb", bufs=2))
    scr_pool = ctx.enter_context(tc.tile_pool(name="scr", bufs=2))
    lgps_pool = ctx.enter_context(tc.tile_pool(name="lgps", bufs=3, space="PSUM"))
    stat_pool = ctx.enter_context(tc.tile_pool(name="stat", bufs=2))
    dram = ctx.enter_context(tc.tile_pool(name="dram", bufs=2, space="DRAM"))

    # per-block state (avg all-reduce) + per-group state (denominators)
    bstate = {k: {} for k in range(N_BB)}
    gstate = {}
    for gi, (g0, g1) in enumerate(DGROUPS):
        gstate[gi] = {
            "g0": g0,
            "g1": g1,
            "den_sb": stat_pool.tile(
                [128, g1 - g0], F32, tag=f"densb{gi}", name=f"densb{gi}"
            ),
        }
    bb2grp = {}
    for gi, (g0, g1) in enumerate(DGROUPS):
        for bb in range(g0, g1):
            bb2grp[bb] = gi

    def stage1_chunk(bb, j, avgT_ps):
        b0 = bb * BB
        v0 = j * VC
        vc = min(VC, Vs - v0)
        oh32 = oh32_pool.tile([128, Cs, VC], F32, tag="oh32")
        nc.sync.dma_start(
            oh32[:, :, :vc], batch[b0 : b0 + BB, :, v0 : v0 + vc]
        )
        ohb = ohb_pool.tile([128, Cs, VC], BF16, tag="ohb")
        if bb >= 6:
            # scalar engine is busy with wave-0's epilogue here
            nc.vector.tensor_copy(ohb[:, :, :vc], oh32[:, :, :vc])
        else:
            nc.vector.tensor_copy(ohb[:, :C_DVE, :vc], oh32[:, :C_DVE, :vc])
            nc.scalar.copy(ohb[:, C_DVE:, :vc], oh32[:, C_DVE:, :vc])
        nt = (vc + 127) // 128
        for t in range(nt):
            toff = t * 128
            tw = min(128, vc - toff)
            g = j * (VC // 128) + t
            sT_ps = sTps_pool.tile([128, BB], F32, tag="sTps")
            for c in range(Cs):
                nc.tensor.matmul(
                    sT_ps[:tw],
                    lhsT=ohb[:, c, toff : toff + tw],
                    rhs=ident[:],
                    start=(c == 0),
                    stop=(c == Cs - 1),
                )
            sT = sT_pool.tile([128, BB], F32, tag="sT")
            nc.vector.tensor_copy(sT[:tw], sT_ps[:tw])
            nc.tensor.matmul(
                avgT_ps[:],
                lhsT=eb[:tw, g, :],
                rhs=sT[:tw],
                start=(g == 0),
                stop=(g == N_TILES - 1),
            )

    def avg_ar(bb):
        """Bounce one block's context-sums to DRAM, AllReduce, read back."""
        cc_in = dram.tile([128, BB], F32, tag="cc_in", bufs=3, name=f"ccin{bb}")
        cc_out = dram.tile(
            [128, BB], F32, tag="cc_out", addr_space="Shared", bufs=3,
            name=f"ccout{bb}",
        )
        nc.sync.dma_start(cc_in[:], bstate[bb]["avg_sb"][:])
        nc.gpsimd.collective_compute(
            "AllReduce",
            mybir.AluOpType.add,
            replica_groups=rg,
            ins=[cc_in.opt()],
            outs=[cc_out.opt()],
        )
        avg_g = avgg_pool.tile([128, BB], F32, tag="avgg", name=f"avgg{bb}")
        nc.gpsimd.dma_start(avg_g[:], cc_out[:])
        bstate[bb]["avg_g"] = avg_g

    def den_ar(gi):
        s = gstate[gi]
        nb = s["g1"] - s["g0"]
        cc_in = dram.tile([128, nb], F32, tag=f"cc2_in{gi}", bufs=1)
        cc_out = dram.tile(
            [128, nb], F32, tag=f"cc2_out{gi}", addr_space="Shared", bufs=1
        )
        nc.sync.dma_start(cc_in[:], s["den_sb"][:])
        nc.gpsimd.collective_compute(
            "AllReduce",
            mybir.AluOpType.add,
            replica_groups=rg,
            ins=[cc_in.opt()],
            outs=[cc_out.opt()],
        )
        den_g = stat_pool.tile([128, nb], F32, tag=f"deng{gi}")
        nc.gpsimd.dma_start(den_g[:], cc_out[:])
        s["den_g"] = den_g

    def stage2a(bb):
        """Logits pass 1: only the exp row-sums survive (scratch discard)."""
        gi = bb2grp[bb]
        gs = gstate[gi]
        avg_g = bstate[bb]["avg_g"]
        sums = stat_pool.tile([128, n_nc], F32, tag="sums")
        for k in range(n_nc):
            n0 = k * NC2
            nw = min(NC2, Vs - n0)
            lg_ps = lgps_pool.tile([128, NC2], F32, tag="lgps")
            nc.tensor.matmul(
                lg_ps[:, :nw],
                lhsT=avg_g[:],
                rhs=wo[:, n0 : n0 + nw],
                start=True,
                stop=True,
            )
            scr = scr_pool.tile([128, NC2], BF16, tag="scr")
            nc.scalar.activation(
                scr[:, :nw],
                lg_ps[:, :nw],
                AF.Exp,
                scale=1.0 / Cs,
                accum_out=sums[:, k : k + 1],
            )
        scr2 = stat_pool.tile([128, n_nc], F32, tag="scr2")
        nc.scalar.activation(
            scr2[:, :n_nc],
            sums[:, :n_nc],
            AF.Copy,
            accum_out=gs["den_sb"][:, bb - gs["g0"] : bb - gs["g0"] + 1],
        )

    def stage2b(gi):
        """Logits pass 2: out = exp(x/C - ln S), written straight to bf16."""
        s = gstate[gi]
        nb = s["g1"] - s["g0"]
        lnS = stat_pool.tile([128, nb], F32, tag=f"lnS{gi}")
        nc.scalar.activation(lnS[:], s["den_g"][:], AF.Ln)
        nlnS = stat_pool.tile([128, nb], F32, tag=f"nlnS{gi}")
        nc.vector.tensor_scalar_mul(nlnS[:], lnS[:], -1.0)
        for bb in range(s["g0"], s["g1"]):
            slot = bb - s["g0"]
            osb = osb_pool.tile([128, VS], BF16, tag="osb")
            for k in range(n_nc):
                n0 = k * NC2
                nw = min(NC2, Vs - n0)
                lg_ps = lgps_pool.tile([128, NC2], F32, tag="lgps")
                nc.tensor.matmul(
                    lg_ps[:, :nw],
                    lhsT=bstate[bb]["avg_g"][:],
                    rhs=wo[:, n0 : n0 + nw],
                    start=True,
                    stop=True,
                )
                nc.scalar.activation(
                    osb[:, n0 : n0 + nw],
                    lg_ps[:, :nw],
                    AF.Exp,
                    scale=1.0 / Cs,
                    bias=nlnS[:, slot : slot + 1],
                )
            b0 = bb * BB
            nc.scalar.dma_start(out[b0 : b0 + BB, :], osb[:])

    # The wo load hides inside the first chunks. Per-block avg ARs fire
    # one window after each block completes; stage-2 consumers sit 2+
    # windows after their producers so nothing head-of-line blocks even
    # with ~20-40 us collectives.
    events = {
        (0, 1): [lambda: nc.sync.dma_start(wo[:], w_out)],
        (1, 1): [lambda: avg_ar(0)],
        (2, 0): [lambda: stage2a(0)],
        (2, 1): [lambda: avg_ar(1)],
        (3, 0): [lambda: stage2a(1)],
        (3, 1): [lambda: avg_ar(2)],
        (4, 0): [lambda: stage2a(2)],
        (4, 1): [lambda: avg_ar(3)],
        (5, 0): [lambda: stage2a(3)],
        (5, 1): [lambda: avg_ar(4)],
        (6, 0): [lambda: stage2a(4)],
        (6, 1): [lambda: avg_ar(5)],
        (6, 2): [lambda: den_ar(0)],
        (6, 3): [lambda: stage2b(0)],
        (7, 0): [lambda: stage2a(5)],
        (7, 1): [lambda: avg_ar(6)],
        (7, 2): [lambda: stage2a(6)],
        (7, 3): [lambda: den_ar(1)],
    }

    for bb in range(N_BB):
        avgT_ps = acc_pool.tile([128, BB], F32, tag="acc")
        for j in range(n_vc):
            stage1_chunk(bb, j, avgT_ps)
            for fn in events.get((bb, j), []):
                fn()
        avg_sb = avgsb_pool.tile([128, BB], F32, tag="avgsb", name=f"avgsb{bb}")
        nc.vector.tensor_copy(avg_sb[:], avgT_ps[:])
        bstate[bb]["avg_sb"] = avg_sb

    stage2b(1)
    avg_ar(7)
    stage2a(7)
    den_ar(2)
    stage2b(2)


def build(num_devices=N_CORES):
    nc = bacc.Bacc(
        "TRN2",
        target_bir_lowering=False,
        debug=False,
        num_devices=num_devices,
        num_swdge_queues=4,
    )
    batch = nc.dram_tensor(
        "batch", [B_FULL, C, VS], F32, kind="ExternalInput"
    ).ap()
    emb = nc.dram_tensor("emb", [VS_PAD, D], F32, kind="ExternalInput").ap()
    w_out = nc.dram_tensor("w_out", [D, VS], F32, kind="ExternalInput").ap()
    out = nc.dram_tensor("out", [B_FULL, VS], BF16, kind="ExternalOutput").ap()
    with tile.TileContext(nc) as tc:
        _cbow_kernel(tc, out, batch, emb, w_out)
    nc.compile()
    return nc


_NC = None


def _build_cached():
    global _NC
    if _NC is None:
        _NC = build()
    return _NC


def _run(batch, emb, w_out, trace=False, **kwargs):
    from concourse.bass_utils import run_bass_kernel_spmd

    nc = _build_cached()
    batch = np.ascontiguousarray(np.asarray(batch, dtype=np.float32))
    emb = np.asarray(emb, dtype=np.float32)
    w_out = np.asarray(w_out, dtype=np.float32)
    in_maps = []
    for i in range(N_CORES):
        v0 = i * VS
        emb_pad = np.zeros((VS_PAD, D), dtype=np.float32)
        emb_pad[:VS] = emb[v0 : v0 + VS]
        in_maps.append(
            {
                "batch": np.ascontiguousarray(batch[:, :, v0 : v0 + VS]),
                "emb": emb_pad,
                "w_out": np.ascontiguousarray(w_out[:, v0 : v0 + VS]),
            }
        )
    res = run_bass_kernel_spmd(
        nc, in_maps, core_ids=list(range(N_CORES)), trace=trace, **kwargs
    )
    out = np.concatenate(
        [r["out"].astype(np.float32) for r in res.results], axis=1
    )
    return out, res


def kernel(batch, emb, w_out):
    out, _ = _run(batch, emb, w_out, trace=False)
    return out
